# revision 30
# baseline (speedup 1.0000x reference)
"""Trainium2 Bass kernel for nn_BiDiBoundaryPredictor.

Math: logits = x @ W + b; a sequential per-timestep scan adds NEG=-10000 to the
boundary logit while a 4-state refractory counter (flag in {0..3}) is nonzero;
outputs are argmax over the masked logits and the log-softmax probability at
the argmax (temperature TAU=2).

Only the logit DIFFERENCE delta = x . (W[:,1]-W[:,0]) + (b1-b0) matters:
  pred_t      = (delta_t + NEG*mask_t) > 0
  gathered_t  = -ln(1 + exp(-|delta_t + NEG*mask_t| / TAU))
  mask_t      = (flag_t > 0), flag driven by bits d_t = delta_t > 0.

Per-core plan (8 cores, 2 sequences each; data-parallel over batch):
  Stream (DMA-bound ~103us): x in 1 MiB tiles; DVE scalar_tensor_tensor with
    accum computes y = sum(x*w) per 128-timestep column -> y_all [128,128].
    Slab PE transposes (y_all -> ys[lane, t]) overlap the stream.
  Tail: the 4-state automaton as an associative map composition, via
    Hillis-Steele scans over map images (4 floats per map): bootstrap 4-step
    blocks with a first-one formula, in-lane HS over 32 blocks, cross-lane HS
    over 64 chunk summaries per sequence, one apply step, closed-form
    per-position masks, then Abs/Exp/Ln epilogue on ACT.
"""
import numpy as np
from contextlib import ExitStack

import concourse.bass as bass
import concourse.tile as tile
from concourse import bacc, mybir
from concourse._compat import with_exitstack
from concourse.masks import make_identity
from concourse.bass_utils import run_bass_kernel_spmd

F32 = mybir.dt.float32
U8 = mybir.dt.uint8
Alu = mybir.AluOpType
Act = mybir.ActivationFunctionType

NEG = -10000.0
TAU = 2.0
N_CORES = 8
BS, L, D = 16, 8192, 512
SEQ_PER_CORE = BS // N_CORES          # 2
ROWS = SEQ_PER_CORE * L               # 16384 x-rows per core
CHUNK = 128                           # timesteps per lane
NLANE = ROWS // CHUNK                 # 128 lanes = (seq, chunk)
LANES_PER_SEQ = L // CHUNK            # 64
TILE_A = 4                            # 128-row chunks per DMA tile (1 MiB)
NBLK = CHUNK // 4                     # 32 four-step blocks per lane
PAD = 5                               # padded block stride for map storage


def _pv(t, n):
    """[P, n, 4] entry view of a PAD-strided map tile (or a slice of it)."""
    return t[:].rearrange("p (b e) -> p b e", e=PAD)[:, :n, 0:4]


def _combine(nc, cmp, Cv, Av, Bv, p, n):
    """Map composition C = A o B (C[blk, s] = A[blk, B[blk, s]]).

    A is the later map, B the earlier; all views [p, n, 4]; cmp is a uint8
    padded scratch tile. 7 DVE ops (compare + predicated copy).
    """
    cv = cmp[:].rearrange("p (b e) -> p b e", e=PAD)[:p, :n, 0:4]
    nc.vector.tensor_copy(Cv, Av[:, :, 0:1].broadcast_to([p, n, 4]))
    for j in (1, 2, 3):
        nc.vector.tensor_scalar(cv, Bv, float(j), None, op0=Alu.is_equal)
        nc.vector.copy_predicated(Cv, cv, Av[:, :, j:j + 1].broadcast_to([p, n, 4]))


@with_exitstack
def _program(ctx: ExitStack, tc: tile.TileContext,
             x_in, w_in, b_in, bnd_out, gth_out):
    nc = tc.nc

    xpool = ctx.enter_context(tc.tile_pool(name="x", bufs=4))
    ppool = ctx.enter_context(tc.tile_pool(name="prod", bufs=2))
    wpool = ctx.enter_context(tc.tile_pool(name="w", bufs=1))
    spool = ctx.enter_context(tc.tile_pool(name="scan", bufs=1))
    psum = ctx.enter_context(tc.tile_pool(name="ps", bufs=2, space="PSUM"))

    wt = wpool.tile([128, D], F32)
    nc.sync.dma_start(wt[:], w_in[:])
    bt = wpool.tile([128, 1], F32)
    nc.sync.dma_start(bt[:], b_in[:])

    # constants + one-time warmups (overlap the stream)
    ident = wpool.tile([128, 128], F32)
    make_identity(nc, ident[:])



    # ---- stream: matvec y[row] = x[row, :] . w  (DVE, under DMA shadow) ----
    y_all = spool.tile([128, NLANE], F32)     # column i = rows [128i, 128(i+1))
    ys = spool.tile([128, CHUNK], F32)        # lane-major, after transpose
    dbits = spool.tile([128, CHUNK], F32)
    c1 = spool.tile([128, NBLK], F32)
    c2 = spool.tile([128, NBLK], F32)
    c12 = spool.tile([128, NBLK], F32)
    # Ln first so bacc picks the natural_log_exp table once for every ACT op;
    # writing into ys makes the warm op a real dependency (scheduled early)
    nc.scalar.activation(ys[:, 0:1], bt[:], Act.Ln, bias=1.0, scale=0.0)
    n_big = ROWS // (128 * TILE_A)            # 32 DMA tiles of 1 MiB
    slab_end = {7: 0, 15: 1, 23: 2, 31: 3}
    for i in range(n_big):
        xt = xpool.tile([128, TILE_A * D], F32)
        src = x_in[i * 128 * TILE_A:(i + 1) * 128 * TILE_A, :] \
            .rearrange("(a p) d -> p a d", a=TILE_A)
        nc.sync.dma_start(xt[:].rearrange("p (a d) -> p a d", a=TILE_A), src)
        for a in range(TILE_A):
            dummy = ppool.tile([128, 1], F32)
            col = i * TILE_A + a
            nc.vector.scalar_tensor_tensor(
                out=dummy[:].broadcast_to([128, D]),
                in0=xt[:, a * D:(a + 1) * D], scalar=1.0, in1=wt[:],
                op0=Alu.mult, op1=Alu.mult,
                accum_out=y_all[:, col:col + 1])
        s = slab_end.get(i)
        if s is not None:
            # 32 lanes finished: fold bias in, transpose to PSUM partition 0
            # (walrus requires it), stage to SBUF, DMA-scatter to lane rows
            sl = slice(32 * s, 32 * (s + 1))
            nc.vector.tensor_scalar(y_all[:, sl], y_all[:, sl], bt[:],
                                    None, op0=Alu.add)
            ps_t = psum.tile([32, 128], F32, tag="pslab")
            nc.tensor.transpose(ps_t[:], y_all[:, sl], ident[:])
            stg = xpool.tile([32, 128], F32, tag="stg", bufs=2)
            nc.scalar.activation(stg[:], ps_t[:], Act.Identity, scale=1.0)
            nc.sync.dma_start(ys[sl, :], stg[:])
            nc.gpsimd.tensor_scalar(dbits[sl, :], ys[sl, :], 0.0, None,
                                    op0=Alu.is_gt)
            # mask-stage precursors, per slab on idle GPSIMD
            dbs = dbits[sl, :].rearrange("p (b e) -> p b e", e=4)
            bqs = [dbs[:, :, q:q + 1] for q in range(4)]
            nc.vector.tensor_tensor(c1[sl, :], bqs[0], bqs[1], op=Alu.max)
            nc.vector.tensor_tensor(c2[sl, :], c1[sl, :], bqs[2], op=Alu.max)
            nc.vector.tensor_tensor(c12[sl, :], bqs[1], bqs[2], op=Alu.max)

    # ---- bootstrap 4-step maps: img[q] = first-one position >= q (else 0) ----
    # H tiles carry NBLK leading identity-map blocks so HS shifts slide into
    # identity instead of needing per-level copies of the unshifted prefix.
    db = dbits[:].rearrange("p (b e) -> p b e", e=4)
    bq = [db[:, :, q:q + 1] for q in range(4)]
    HW_ = 2 * NBLK
    H0 = spool.tile([128, HW_ * PAD], F32)
    H1 = spool.tile([128, HW_ * PAD], F32)
    idmap = spool.tile([128, LANES_PER_SEQ * PAD], mybir.dt.int32)
    nc.gpsimd.iota(
        idmap[:].rearrange("p (b e) -> p b e", e=PAD)[:, :, 0:4],
        pattern=[[0, LANES_PER_SEQ], [1, 4]], channel_multiplier=0)
    for Ht in (H0, H1):
        nc.gpsimd.tensor_copy(
            _pv(Ht, NBLK),
            idmap[:].rearrange("p (b e) -> p b e", e=PAD)[:, :NBLK, 0:4])
    t1 = spool.tile([128, NBLK], F32)
    t2 = spool.tile([128, NBLK], F32)
    hv = H0[:].rearrange("p (b e) -> p b e", e=PAD)[:, NBLK:, :]
    i3, i2, i1, i0 = (hv[:, :, j:j + 1] for j in (3, 2, 1, 0))
    nc.vector.tensor_scalar(i3, bq[3], 3.0, None, op0=Alu.mult)
    nc.vector.scalar_tensor_tensor(out=t2[:], in0=i3, scalar=2.0, in1=bq[2],
                                   op0=Alu.subtract, op1=Alu.mult)
    nc.vector.tensor_tensor(i2, i3, t2[:], op=Alu.subtract)
    nc.vector.scalar_tensor_tensor(out=t2[:], in0=i2, scalar=1.0, in1=bq[1],
                                   op0=Alu.subtract, op1=Alu.mult)
    nc.vector.tensor_tensor(i1, i2, t2[:], op=Alu.subtract)
    nc.vector.tensor_tensor(t2[:], i1, bq[0], op=Alu.mult)
    nc.vector.tensor_tensor(i0, i1, t2[:], op=Alu.subtract)

    # ---- in-lane Hillis-Steele over 32 blocks (inclusive prefixes) ----
    cmp_s = spool.tile([128, NBLK * PAD], U8)
    cur, nxt = H0, H1
    for k in (1, 2, 4, 8, 16):
        cv = _pv(nxt, HW_)[:, NBLK:, :]
        av = _pv(cur, HW_)[:, NBLK:, :]
        bv = _pv(cur, HW_)[:, NBLK - k:HW_ - k, :]
        _combine(nc, cmp_s, cv, av, bv, 128, NBLK)
        cur, nxt = nxt, cur
    Hf = cur                                   # inclusive prefixes at [NBLK:]

    # ---- cross-lane HS over the 64 chunk summaries of each sequence ----
    M = LANES_PER_SEQ
    Sc = spool.tile([2, M * 4], F32)
    for s in range(SEQ_PER_CORE):
        nc.sync.dma_start(
            Sc[s:s + 1, :],
            Hf[64 * s:64 * (s + 1), (HW_ - 1) * PAD:(HW_ - 1) * PAD + 4])
    TW = 2 * M
    T0 = spool.tile([2, TW * PAD], F32)
    T1 = spool.tile([2, TW * PAD], F32)
    cmp2 = spool.tile([2, TW * PAD], U8)
    idm2 = idmap[:].rearrange("p (b e) -> p b e", e=PAD)[0:2, :M, 0:4]
    for Tt in (T0, T1):
        nc.gpsimd.tensor_copy(_pv(Tt, M)[:2], idm2)
    nc.vector.tensor_copy(_pv(T0, TW)[:2, M:, :],
                          Sc[:].rearrange("p (b e) -> p b e", e=4))
    cur, nxt = T0, T1
    for k in (1, 2, 4, 8, 16, 32):
        cv = _pv(nxt, TW)[:2, M:, :]
        av = _pv(cur, TW)[:2, M:, :]
        bv = _pv(cur, TW)[:2, M - k:TW - k, :]
        _combine(nc, cmp2, cv, av, bv, 2, M)
        cur, nxt = nxt, cur
    Sf = cur                                   # inclusive chunk prefixes

    # lane-entry STATES (entry 0 only): lane l gets inclusive[l-1][0]; lane 0 -> 0
    El0 = spool.tile([128, 1], F32)
    nc.gpsimd.memset(El0[:], 0.0)
    for s in range(SEQ_PER_CORE):
        nc.sync.dma_start(
            El0[64 * s + 1:64 * (s + 1), 0:1],
            _pv(Sf, TW)[s:s + 1, M:TW - 1, 0:1])

    # ---- apply: block-entry state per 4-block: sblk[b] = HfExcl[b][El0] ----
    sblk = spool.tile([128, NBLK], F32)
    nc.vector.tensor_copy(sblk[:, 0:1], El0[:])
    av = _pv(Hf, HW_)[:, NBLK:HW_ - 1, :]      # in-lane inclusive, blocks 0..30
    cm1 = spool.tile([128, 1], U8)
    nc.vector.tensor_copy(sblk[:, 1:NBLK], av[:, :, 0:1])   # El0==0 case
    for j in (1, 2, 3):
        nc.vector.tensor_scalar(cm1[:], El0[:], float(j), None, op0=Alu.is_equal)
        nc.vector.copy_predicated(sblk[:, 1:NBLK],
                                  cm1[:].broadcast_to([128, NBLK - 1]),
                                  av[:, :, j:j + 1])

    # ---- per-position masks from block-entry state s and bits ----
    mask = spool.tile([128, CHUNK], F32)
    mv = mask[:].rearrange("p (b e) -> p b e", e=4)
    mq = [mv[:, :, q:q + 1] for q in range(4)]
    e0 = spool.tile([128, NBLK], F32)
    e1 = spool.tile([128, NBLK], F32)
    e2 = spool.tile([128, NBLK], F32)
    u = spool.tile([128, NBLK], F32)
    u2 = spool.tile([128, NBLK], F32)
    nc.vector.tensor_scalar(e0[:], sblk[:], 0.0, None, op0=Alu.is_equal)
    nc.vector.tensor_scalar(e1[:], sblk[:], 1.0, None, op0=Alu.is_equal)
    nc.vector.tensor_scalar(e2[:], sblk[:], 2.0, None, op0=Alu.is_equal)
    nc.vector.tensor_scalar(mq[0], sblk[:], 0.0, None, op0=Alu.is_gt)
    # m1 = max(s>1, e0*b0)
    nc.vector.tensor_tensor(u[:], e0[:], bq[0], op=Alu.mult)
    nc.vector.scalar_tensor_tensor(out=mq[1], in0=sblk[:], scalar=1.0,
                                   in1=u[:], op0=Alu.is_gt, op1=Alu.max)
    # m2 = max(s>2, e0*c1, e1*b1)
    nc.vector.tensor_tensor(u[:], e0[:], c1[:], op=Alu.mult)
    nc.vector.scalar_tensor_tensor(out=mq[2], in0=sblk[:], scalar=2.0,
                                   in1=u[:], op0=Alu.is_gt, op1=Alu.max)
    nc.vector.tensor_tensor(u[:], e1[:], bq[1], op=Alu.mult)
    nc.vector.tensor_tensor(mq[2], mv[:, :, 2:3], u[:], op=Alu.max)
    # m3 = max(e0*c2, e1*c12, e2*b2)
    nc.vector.tensor_tensor(u[:], e0[:], c2[:], op=Alu.mult)
    nc.vector.tensor_tensor(u2[:], e1[:], c12[:], op=Alu.mult)
    nc.vector.tensor_tensor(mq[3], u[:], u2[:], op=Alu.max)
    nc.vector.tensor_tensor(u[:], e2[:], bq[2], op=Alu.mult)
    nc.vector.tensor_tensor(mq[3], mv[:, :, 3:4], u[:], op=Alu.max)

    # ---- epilogue ----
    dm = spool.tile([128, CHUNK], F32)
    nc.vector.scalar_tensor_tensor(
        out=dm[:], in0=mask[:], scalar=NEG, in1=ys[:],
        op0=Alu.mult, op1=Alu.add)
    bnd = spool.tile([128, CHUNK], F32)
    nc.vector.tensor_scalar(bnd[:], dm[:], 0.0, None, op0=Alu.is_gt)
    ab = spool.tile([128, CHUNK], F32)
    nc.scalar.activation(ab[:], dm[:], Act.Abs)
    ex = spool.tile([128, CHUNK], F32)
    nc.scalar.activation(ex[:], ab[:], Act.Exp, scale=-1.0 / TAU)
    lg = spool.tile([128, CHUNK], F32)
    nc.scalar.activation(lg[:], ex[:], Act.Ln, bias=1.0)
    gth = spool.tile([128, CHUNK], F32)
    nc.scalar.activation(gth[:], lg[:], Act.Copy, scale=-1.0)

    bdst = bnd_out.rearrange("s (c w) -> (s c) w", w=CHUNK)
    gdst = gth_out.rearrange("s (c w) -> (s c) w", w=CHUNK)
    nc.sync.dma_start(bdst, bnd[:])
    nc.sync.dma_start(gdst, gth[:])


def build_program():
    nc = bacc.Bacc()
    x_in = nc.declare_dram_parameter("x", [ROWS, D], F32, isOutput=False)
    w_in = nc.declare_dram_parameter("w", [128, D], F32, isOutput=False)
    b_in = nc.declare_dram_parameter("bias", [128, 1], F32, isOutput=False)
    bnd_out = nc.declare_dram_parameter("bnd", [SEQ_PER_CORE, L], F32, isOutput=True)
    gth_out = nc.declare_dram_parameter("gth", [SEQ_PER_CORE, L], F32, isOutput=True)
    with tile.TileContext(nc) as tc:
        _program(tc, x_in[:], w_in[:], b_in[:], bnd_out[:], gth_out[:])
    nc.compile()
    return nc


_NC_CACHE = None


def kernel(x, label, W, b, _trace=False, _tmpdir=None):
    global _NC_CACHE
    x = np.ascontiguousarray(np.asarray(x, dtype=np.float32))
    W = np.asarray(W, dtype=np.float32)
    b = np.asarray(b, dtype=np.float32)
    wd = np.ascontiguousarray(np.repeat((W[:, 1] - W[:, 0])[None, :], 128, axis=0))
    bd = np.full((128, 1), np.float32(b[1] - b[0]), dtype=np.float32)

    if _NC_CACHE is None:
        _NC_CACHE = build_program()
    nc = _NC_CACHE

    in_maps = []
    for c in range(N_CORES):
        shard = x[c * SEQ_PER_CORE:(c + 1) * SEQ_PER_CORE].reshape(ROWS, D)
        in_maps.append({"x": np.ascontiguousarray(shard), "w": wd, "bias": bd})

    res = run_bass_kernel_spmd(nc, in_maps, list(range(N_CORES)),
                               trace=_trace, tmpdir=_tmpdir)
    boundaries = np.concatenate(
        [res.results[c]["bnd"] for c in range(N_CORES)], axis=0)
    gathered = np.concatenate(
        [res.results[c]["gth"] for c in range(N_CORES)], axis=0)[..., None]
    out = (boundaries.astype(np.float32), gathered.astype(np.float32))
    if _trace:
        return out, res
    return out


# revision 36
# speedup vs baseline: 1.0792x; 1.0792x over previous
"""Trainium2 Bass kernel for nn_BiDiBoundaryPredictor.

Math: logits = x @ W + b; a sequential per-timestep scan adds NEG=-10000 to the
boundary logit while a 4-state refractory counter (flag in {0..3}) is nonzero;
outputs are argmax over the masked logits and the log-softmax probability at
the argmax (temperature TAU=2).

Only the logit DIFFERENCE delta = x . (W[:,1]-W[:,0]) + (b1-b0) matters:
  pred_t      = (delta_t + NEG*mask_t) > 0
  gathered_t  = -ln(1 + exp(-|delta_t + NEG*mask_t| / TAU))
  mask_t      = (flag_t > 0), flag driven by bits d_t = delta_t > 0.

Per-core plan (8 cores, 2 sequences each; data-parallel over batch):
  Stream (DMA-bound ~103us): x in 1 MiB tiles; DVE scalar_tensor_tensor with
    accum computes y = sum(x*w) per 128-timestep column -> y_all [128,128].
    Slab PE transposes (y_all -> ys[lane, t]) overlap the stream.
  Tail: the 4-state automaton as an associative map composition, via
    Hillis-Steele scans over map images (4 floats per map): bootstrap 4-step
    blocks with a first-one formula, in-lane HS over 32 blocks, cross-lane HS
    over 64 chunk summaries per sequence, one apply step, closed-form
    per-position masks, then Abs/Exp/Ln epilogue on ACT.
"""
import numpy as np
from contextlib import ExitStack

import concourse.bass as bass
import concourse.tile as tile
from concourse import bacc, mybir
from concourse._compat import with_exitstack
from concourse.masks import make_identity
from concourse.bass_utils import run_bass_kernel_spmd

F32 = mybir.dt.float32
U8 = mybir.dt.uint8
Alu = mybir.AluOpType
Act = mybir.ActivationFunctionType

NEG = -10000.0
TAU = 2.0
N_CORES = 8
BS, L, D = 16, 8192, 512
SEQ_PER_CORE = BS // N_CORES          # 2
ROWS = SEQ_PER_CORE * L               # 16384 x-rows per core
CHUNK = 128                           # timesteps per lane
NLANE = ROWS // CHUNK                 # 128 lanes = (seq, chunk)
LANES_PER_SEQ = L // CHUNK            # 64
TILE_A = 4                            # 128-row chunks per DMA tile (1 MiB)
NBLK = CHUNK // 4                     # 32 four-step blocks per lane
PAD = 5                               # padded block stride for map storage


def _pv(t, n):
    """[P, n, 4] entry view of a PAD-strided map tile (or a slice of it)."""
    return t[:].rearrange("p (b e) -> p b e", e=PAD)[:, :n, 0:4]


def _combine(nc, tjt, Cv, Av, Bv, p, n):
    """Map composition C = A o B (C[blk, s] = A[blk, B[blk, s]]).

    Arithmetic gather C = sum_j (B==j)*A[j]; A later, B earlier; views
    [p, n, 4]; tjt an f32 padded scratch tile. 7 DVE ops (4 stt + 3 tt).
    """
    tv = tjt[:].rearrange("p (b e) -> p b e", e=PAD)[:p, :n, 0:4]
    nc.vector.scalar_tensor_tensor(
        out=Cv, in0=Bv, scalar=0.0, in1=Av[:, :, 0:1].broadcast_to([p, n, 4]),
        op0=Alu.is_equal, op1=Alu.mult)
    for j in (1, 2, 3):
        nc.vector.scalar_tensor_tensor(
            out=tv, in0=Bv, scalar=float(j),
            in1=Av[:, :, j:j + 1].broadcast_to([p, n, 4]),
            op0=Alu.is_equal, op1=Alu.mult)
        nc.vector.tensor_tensor(Cv, Cv, tv, op=Alu.add)


@with_exitstack
def _program(ctx: ExitStack, tc: tile.TileContext,
             x_in, w_in, b_in, bnd_out, gth_out):
    nc = tc.nc

    xpool = ctx.enter_context(tc.tile_pool(name="x", bufs=4))
    ppool = ctx.enter_context(tc.tile_pool(name="prod", bufs=2))
    wpool = ctx.enter_context(tc.tile_pool(name="w", bufs=1))
    spool = ctx.enter_context(tc.tile_pool(name="scan", bufs=1))
    psum = ctx.enter_context(tc.tile_pool(name="ps", bufs=2, space="PSUM"))

    wt = wpool.tile([128, D], F32)
    nc.sync.dma_start(wt[:], w_in[:])
    bt = wpool.tile([128, 1], F32)
    nc.sync.dma_start(bt[:], b_in[:])

    # constants + one-time warmups (overlap the stream)
    ident = wpool.tile([128, 128], F32)
    make_identity(nc, ident[:])



    # ---- stream: matvec y[row] = x[row, :] . w  (DVE, under DMA shadow) ----
    y_all = spool.tile([128, NLANE], F32)     # column i = rows [128i, 128(i+1))
    ys = spool.tile([128, CHUNK], F32)        # lane-major, after transpose
    dbits = spool.tile([128, CHUNK], F32)
    c1 = spool.tile([128, NBLK], F32)
    c2 = spool.tile([128, NBLK], F32)
    c12 = spool.tile([128, NBLK], F32)
    nbt = spool.tile([128, 1], F32)
    nc.gpsimd.tensor_scalar(nbt[:], bt[:], -1.0, None, op0=Alu.mult)
    # Ln first so bacc picks the natural_log_exp table once for every ACT op;
    # c1 gets overwritten by slab ops, making this a real early dependency
    nc.scalar.activation(c1[:, 0:1], bt[:], Act.Ln, bias=1.0, scale=0.0)
    n_big = ROWS // (128 * TILE_A)            # 32 DMA tiles of 1 MiB
    slab_end = {7: 0, 15: 1, 23: 2, 31: 3}
    for i in range(n_big):
        xt = xpool.tile([128, TILE_A * D], F32)
        src = x_in[i * 128 * TILE_A:(i + 1) * 128 * TILE_A, :] \
            .rearrange("(a p) d -> p a d", a=TILE_A)
        nc.sync.dma_start(xt[:].rearrange("p (a d) -> p a d", a=TILE_A), src)
        for a in range(TILE_A):
            dummy = ppool.tile([128, 1], F32)
            col = i * TILE_A + a
            nc.vector.scalar_tensor_tensor(
                out=dummy[:].broadcast_to([128, D]),
                in0=xt[:, a * D:(a + 1) * D], scalar=1.0, in1=wt[:],
                op0=Alu.mult, op1=Alu.mult,
                accum_out=y_all[:, col:col + 1])
        s = slab_end.get(i)
        if s is not None:
            # 32 lanes finished: transpose to PSUM partition 0 (walrus
            # requires it), stage to SBUF, DMA-scatter to lane rows.
            # Bias is folded later via per-partition AP scalars.
            sl = slice(32 * s, 32 * (s + 1))
            ps_t = psum.tile([32, 128], F32, tag="pslab")
            nc.tensor.transpose(ps_t[:], y_all[:, sl], ident[:])
            stg = xpool.tile([32, 128], F32, tag="stg", bufs=2)
            nc.scalar.activation(stg[:], ps_t[:], Act.Identity, scale=1.0)
            nc.sync.dma_start(ys[sl, :], stg[:])
            nc.vector.tensor_scalar(dbits[sl, :], ys[sl, :], nbt[sl, :], None,
                                    op0=Alu.is_gt)
            # mask-stage precursors, per slab on idle GPSIMD
            dbs = dbits[sl, :].rearrange("p (b e) -> p b e", e=4)
            bqs = [dbs[:, :, q:q + 1] for q in range(4)]
            nc.vector.tensor_tensor(c1[sl, :], bqs[0], bqs[1], op=Alu.max)
            nc.vector.tensor_tensor(c2[sl, :], c1[sl, :], bqs[2], op=Alu.max)
            nc.vector.tensor_tensor(c12[sl, :], bqs[1], bqs[2], op=Alu.max)

    # ---- bootstrap 4-step maps: img[q] = first-one position >= q (else 0) ----
    # H tiles carry NBLK leading identity-map blocks so HS shifts slide into
    # identity instead of needing per-level copies of the unshifted prefix.
    db = dbits[:].rearrange("p (b e) -> p b e", e=4)
    bq = [db[:, :, q:q + 1] for q in range(4)]
    HW_ = 2 * NBLK
    H0 = spool.tile([128, HW_ * PAD], F32)
    H1 = spool.tile([128, HW_ * PAD], F32)
    idmap = spool.tile([128, LANES_PER_SEQ * PAD], mybir.dt.int32)
    nc.gpsimd.iota(
        idmap[:].rearrange("p (b e) -> p b e", e=PAD)[:, :, 0:4],
        pattern=[[0, LANES_PER_SEQ], [1, 4]], channel_multiplier=0)
    for Ht in (H0, H1):
        nc.gpsimd.tensor_copy(
            _pv(Ht, NBLK),
            idmap[:].rearrange("p (b e) -> p b e", e=PAD)[:, :NBLK, 0:4])
    t1 = spool.tile([128, NBLK], F32)
    t2 = spool.tile([128, NBLK], F32)
    hv = H0[:].rearrange("p (b e) -> p b e", e=PAD)[:, NBLK:, :]
    i3, i2, i1, i0 = (hv[:, :, j:j + 1] for j in (3, 2, 1, 0))
    nc.vector.tensor_scalar(i3, bq[3], 3.0, None, op0=Alu.mult)
    nc.vector.scalar_tensor_tensor(out=t2[:], in0=i3, scalar=2.0, in1=bq[2],
                                   op0=Alu.subtract, op1=Alu.mult)
    nc.vector.tensor_tensor(i2, i3, t2[:], op=Alu.subtract)
    nc.vector.scalar_tensor_tensor(out=t2[:], in0=i2, scalar=1.0, in1=bq[1],
                                   op0=Alu.subtract, op1=Alu.mult)
    nc.vector.tensor_tensor(i1, i2, t2[:], op=Alu.subtract)
    nc.vector.tensor_tensor(t2[:], i1, bq[0], op=Alu.mult)
    nc.vector.tensor_tensor(i0, i1, t2[:], op=Alu.subtract)

    # ---- in-lane Hillis-Steele over 32 blocks (inclusive prefixes) ----
    cmp_s = spool.tile([128, NBLK * PAD], F32)
    cur, nxt = H0, H1
    for k in (1, 2, 4, 8, 16):
        cv = _pv(nxt, HW_)[:, NBLK:, :]
        av = _pv(cur, HW_)[:, NBLK:, :]
        bv = _pv(cur, HW_)[:, NBLK - k:HW_ - k, :]
        _combine(nc, cmp_s, cv, av, bv, 128, NBLK)
        cur, nxt = nxt, cur
    Hf = cur                                   # inclusive prefixes at [NBLK:]

    # ---- cross-lane HS over the 64 chunk summaries of each sequence ----
    M = LANES_PER_SEQ
    Sc = spool.tile([2, M * 4], F32)
    for s in range(SEQ_PER_CORE):
        nc.sync.dma_start(
            Sc[s:s + 1, :],
            Hf[64 * s:64 * (s + 1), (HW_ - 1) * PAD:(HW_ - 1) * PAD + 4])
    TW = 2 * M
    T0 = spool.tile([2, TW * PAD], F32)
    T1 = spool.tile([2, TW * PAD], F32)
    cmp2 = spool.tile([2, TW * PAD], F32)
    idm2 = idmap[:].rearrange("p (b e) -> p b e", e=PAD)[0:2, :M, 0:4]
    for Tt in (T0, T1):
        nc.gpsimd.tensor_copy(_pv(Tt, M)[:2], idm2)
    nc.vector.tensor_copy(_pv(T0, TW)[:2, M:, :],
                          Sc[:].rearrange("p (b e) -> p b e", e=4))
    cur, nxt = T0, T1
    for k in (1, 2, 4, 8, 16, 32):
        cv = _pv(nxt, TW)[:2, M:, :]
        av = _pv(cur, TW)[:2, M:, :]
        bv = _pv(cur, TW)[:2, M - k:TW - k, :]
        _combine(nc, cmp2, cv, av, bv, 2, M)
        cur, nxt = nxt, cur
    Sf = cur                                   # inclusive chunk prefixes

    # lane-entry STATES (entry 0 only): lane l gets inclusive[l-1][0]; lane 0 -> 0
    El0 = spool.tile([128, 1], F32)
    nc.gpsimd.memset(El0[:], 0.0)
    for s in range(SEQ_PER_CORE):
        nc.sync.dma_start(
            El0[64 * s + 1:64 * (s + 1), 0:1],
            _pv(Sf, TW)[s:s + 1, M:TW - 1, 0:1])

    # ---- apply: block-entry state per 4-block: sblk[b] = HfExcl[b][El0] ----
    sblk = spool.tile([128, NBLK], F32)
    nc.vector.tensor_copy(sblk[:, 0:1], El0[:])
    av = _pv(Hf, HW_)[:, NBLK:HW_ - 1, :]      # in-lane inclusive, blocks 0..30
    cm1 = spool.tile([128, 1], U8)
    nc.vector.tensor_copy(sblk[:, 1:NBLK], av[:, :, 0:1])   # El0==0 case
    for j in (1, 2, 3):
        nc.vector.tensor_scalar(cm1[:], El0[:], float(j), None, op0=Alu.is_equal)
        nc.vector.copy_predicated(sblk[:, 1:NBLK],
                                  cm1[:].broadcast_to([128, NBLK - 1]),
                                  av[:, :, j:j + 1])

    # ---- per-position masks from block-entry state s and bits ----
    mask = spool.tile([128, CHUNK], F32)
    mv = mask[:].rearrange("p (b e) -> p b e", e=4)
    mq = [mv[:, :, q:q + 1] for q in range(4)]
    e0 = spool.tile([128, NBLK], F32)
    e1 = spool.tile([128, NBLK], F32)
    e2 = spool.tile([128, NBLK], F32)
    u = spool.tile([128, NBLK], F32)
    u2 = spool.tile([128, NBLK], F32)
    nc.vector.tensor_scalar(e0[:], sblk[:], 0.0, None, op0=Alu.is_equal)
    nc.vector.tensor_scalar(e1[:], sblk[:], 1.0, None, op0=Alu.is_equal)
    nc.vector.tensor_scalar(e2[:], sblk[:], 2.0, None, op0=Alu.is_equal)
    nc.vector.tensor_scalar(mq[0], sblk[:], 0.0, None, op0=Alu.is_gt)
    # m1 = max(s>1, e0*b0)
    nc.vector.tensor_tensor(u[:], e0[:], bq[0], op=Alu.mult)
    nc.vector.scalar_tensor_tensor(out=mq[1], in0=sblk[:], scalar=1.0,
                                   in1=u[:], op0=Alu.is_gt, op1=Alu.max)
    # m2 = max(s>2, e0*c1, e1*b1)
    nc.vector.tensor_tensor(u[:], e0[:], c1[:], op=Alu.mult)
    nc.vector.scalar_tensor_tensor(out=mq[2], in0=sblk[:], scalar=2.0,
                                   in1=u[:], op0=Alu.is_gt, op1=Alu.max)
    nc.vector.tensor_tensor(u[:], e1[:], bq[1], op=Alu.mult)
    nc.vector.tensor_tensor(mq[2], mv[:, :, 2:3], u[:], op=Alu.max)
    # m3 = max(e0*c2, e1*c12, e2*b2)
    nc.vector.tensor_tensor(u[:], e0[:], c2[:], op=Alu.mult)
    nc.vector.tensor_tensor(u2[:], e1[:], c12[:], op=Alu.mult)
    nc.vector.tensor_tensor(mq[3], u[:], u2[:], op=Alu.max)
    nc.vector.tensor_tensor(u[:], e2[:], bq[2], op=Alu.mult)
    nc.vector.tensor_tensor(mq[3], mv[:, :, 3:4], u[:], op=Alu.max)

    # ---- epilogue ----
    dm = spool.tile([128, CHUNK], F32)
    nc.vector.scalar_tensor_tensor(
        out=dm[:], in0=mask[:], scalar=NEG, in1=ys[:],
        op0=Alu.mult, op1=Alu.add)
    bnd = spool.tile([128, CHUNK], F32)
    nc.vector.tensor_scalar(bnd[:], dm[:], nbt[:], None, op0=Alu.is_gt)
    ab = spool.tile([128, CHUNK], F32)
    nc.scalar.activation(ab[:], dm[:], Act.Abs, bias=bt[:])
    ex = spool.tile([128, CHUNK], F32)
    nc.scalar.activation(ex[:], ab[:], Act.Exp, scale=-1.0 / TAU)
    lg = spool.tile([128, CHUNK], F32)
    nc.scalar.activation(lg[:], ex[:], Act.Ln, bias=1.0)
    gth = spool.tile([128, CHUNK], F32)
    nc.scalar.activation(gth[:], lg[:], Act.Copy, scale=-1.0)

    bdst = bnd_out.rearrange("s (c w) -> (s c) w", w=CHUNK)
    gdst = gth_out.rearrange("s (c w) -> (s c) w", w=CHUNK)
    nc.sync.dma_start(bdst, bnd[:])
    nc.sync.dma_start(gdst, gth[:])


def build_program():
    nc = bacc.Bacc()
    x_in = nc.declare_dram_parameter("x", [ROWS, D], F32, isOutput=False)
    w_in = nc.declare_dram_parameter("w", [128, D], F32, isOutput=False)
    b_in = nc.declare_dram_parameter("bias", [128, 1], F32, isOutput=False)
    bnd_out = nc.declare_dram_parameter("bnd", [SEQ_PER_CORE, L], F32, isOutput=True)
    gth_out = nc.declare_dram_parameter("gth", [SEQ_PER_CORE, L], F32, isOutput=True)
    with tile.TileContext(nc) as tc:
        _program(tc, x_in[:], w_in[:], b_in[:], bnd_out[:], gth_out[:])
    nc.compile()
    return nc


_NC_CACHE = None


def kernel(x, label, W, b, _trace=False, _tmpdir=None):
    global _NC_CACHE
    x = np.ascontiguousarray(np.asarray(x, dtype=np.float32))
    W = np.asarray(W, dtype=np.float32)
    b = np.asarray(b, dtype=np.float32)
    wd = np.ascontiguousarray(np.repeat((W[:, 1] - W[:, 0])[None, :], 128, axis=0))
    bd = np.full((128, 1), np.float32(b[1] - b[0]), dtype=np.float32)

    if _NC_CACHE is None:
        _NC_CACHE = build_program()
    nc = _NC_CACHE

    in_maps = []
    for c in range(N_CORES):
        shard = x[c * SEQ_PER_CORE:(c + 1) * SEQ_PER_CORE].reshape(ROWS, D)
        in_maps.append({"x": np.ascontiguousarray(shard), "w": wd, "bias": bd})

    res = run_bass_kernel_spmd(nc, in_maps, list(range(N_CORES)),
                               trace=_trace, tmpdir=_tmpdir)
    boundaries = np.concatenate(
        [res.results[c]["bnd"] for c in range(N_CORES)], axis=0)
    gathered = np.concatenate(
        [res.results[c]["gth"] for c in range(N_CORES)], axis=0)[..., None]
    out = (boundaries.astype(np.float32), gathered.astype(np.float32))
    if _trace:
        return out, res
    return out


# revision 46
# speedup vs baseline: 1.1446x; 1.0606x over previous
"""Trainium2 Bass kernel for nn_BiDiBoundaryPredictor.

Math: logits = x @ W + b; a sequential per-timestep scan adds NEG=-10000 to the
boundary logit while a 4-state refractory counter (flag in {0..3}) is nonzero;
outputs are argmax over the masked logits and the log-softmax probability at
the argmax (temperature TAU=2).

Only the logit DIFFERENCE delta = x . (W[:,1]-W[:,0]) + (b1-b0) matters:
  pred_t      = (delta_t + NEG*mask_t) > 0
  gathered_t  = -ln(1 + exp(-|delta_t + NEG*mask_t| / TAU))
  mask_t      = (flag_t > 0), flag driven by bits d_t = delta_t > 0.

Per-core plan (8 cores, 2 sequences each; data-parallel over batch):
  Stream (DMA-bound ~103us): x in 1 MiB tiles; DVE scalar_tensor_tensor with
    accum computes y = sum(x*w) per 128-timestep column -> y_all [128,128].
    Slab PE transposes (y_all -> ys[lane, t]) overlap the stream.
  Tail: the 4-state automaton as an associative map composition, via
    Hillis-Steele scans over map images (4 floats per map): bootstrap 4-step
    blocks with a first-one formula, in-lane HS over 32 blocks, cross-lane HS
    over 64 chunk summaries per sequence, one apply step, closed-form
    per-position masks, then Abs/Exp/Ln epilogue on ACT.
"""
import numpy as np
from contextlib import ExitStack

import concourse.bass as bass
import concourse.tile as tile
from concourse import bacc, mybir
from concourse._compat import with_exitstack
from concourse.masks import make_identity
from concourse.bass_utils import run_bass_kernel_spmd

F32 = mybir.dt.float32
U8 = mybir.dt.uint8
Alu = mybir.AluOpType
Act = mybir.ActivationFunctionType

NEG = -10000.0
TAU = 2.0
N_CORES = 8
BS, L, D = 16, 8192, 512
SEQ_PER_CORE = BS // N_CORES          # 2
ROWS = SEQ_PER_CORE * L               # 16384 x-rows per core
CHUNK = 128                           # timesteps per lane
NLANE = ROWS // CHUNK                 # 128 lanes = (seq, chunk)
LANES_PER_SEQ = L // CHUNK            # 64
TILE_A = 4                            # 128-row chunks per DMA tile (1 MiB)
NBLK = CHUNK // 4                     # 32 four-step blocks per lane
PAD = 5                               # padded block stride for map storage


def _pv(t, n):
    """[P, n, 4] entry view of a PAD-strided map tile (or a slice of it)."""
    return t[:].rearrange("p (b e) -> p b e", e=PAD)[:, :n, 0:4]


def _combine(nc, cmp, Cv, Av, Bv, p, n):
    """Map composition C = A o B (C[blk, s] = A[blk, B[blk, s]]).

    A is the later map, B the earlier; views [p, n, 4]; cmp a uint8 padded
    scratch. 7 DVE ops (compare + predicated copy).
    """
    cv = cmp[:].rearrange("p (b e) -> p b e", e=PAD)[:p, :n, 0:4]
    nc.vector.tensor_copy(Cv, Av[:, :, 0:1].broadcast_to([p, n, 4]))
    for j in (1, 2, 3):
        nc.vector.tensor_scalar(cv, Bv, float(j), None, op0=Alu.is_equal)
        nc.vector.copy_predicated(Cv, cv, Av[:, :, j:j + 1].broadcast_to([p, n, 4]))


@with_exitstack
def _program(ctx: ExitStack, tc: tile.TileContext,
             x_in, w_in, b_in, bnd_out, gth_out):
    nc = tc.nc

    xpool = ctx.enter_context(tc.tile_pool(name="x", bufs=6))
    ppool = ctx.enter_context(tc.tile_pool(name="prod", bufs=2))
    wpool = ctx.enter_context(tc.tile_pool(name="w", bufs=1))
    spool = ctx.enter_context(tc.tile_pool(name="scan", bufs=1))
    psum = ctx.enter_context(tc.tile_pool(name="ps", bufs=2, space="PSUM"))

    wt = wpool.tile([128, D], F32)
    nc.sync.dma_start(wt[:], w_in[:])
    bt = wpool.tile([128, 1], F32)
    nc.sync.dma_start(bt[:], b_in[:])

    # constants + one-time warmups (overlap the stream)
    ident = wpool.tile([128, 128], F32)
    make_identity(nc, ident[:])



    # ---- stream: matvec y[row] = x[row, :] . w  (DVE, under DMA shadow) ----
    y_all = spool.tile([128, NLANE], F32)     # column i = rows [128i, 128(i+1))
    ys = spool.tile([128, CHUNK], F32)        # lane-major, after transpose
    dbits = spool.tile([128, CHUNK], F32)
    c1 = spool.tile([128, NBLK], F32)
    c2 = spool.tile([128, NBLK], F32)
    c12 = spool.tile([128, NBLK], F32)
    nbt = spool.tile([128, 1], F32)
    nc.gpsimd.tensor_scalar(nbt[:], bt[:], -1.0, None, op0=Alu.mult)
    # Ln first so bacc picks the natural_log_exp table once for every ACT op;
    # c1 gets overwritten by slab ops, making this a real early dependency
    nc.scalar.activation(c1[:, 0:1], bt[:], Act.Ln, bias=1.0, scale=0.0)
    n_big = ROWS // (128 * TILE_A)            # 32 DMA tiles of 1 MiB
    slab_end = {7: 0, 15: 1, 23: 2, 31: 3}
    for i in range(n_big):
        xt = xpool.tile([128, TILE_A * D], F32)
        src = x_in[i * 128 * TILE_A:(i + 1) * 128 * TILE_A, :] \
            .rearrange("(a p) d -> p a d", a=TILE_A)
        nc.sync.dma_start(xt[:].rearrange("p (a d) -> p a d", a=TILE_A), src)
        for a in range(TILE_A):
            dummy = ppool.tile([128, 1], F32)
            col = i * TILE_A + a
            nc.vector.scalar_tensor_tensor(
                out=dummy[:].broadcast_to([128, D]),
                in0=xt[:, a * D:(a + 1) * D], scalar=1.0, in1=wt[:],
                op0=Alu.mult, op1=Alu.mult,
                accum_out=y_all[:, col:col + 1])
        s = slab_end.get(i)
        if s is not None:
            # 32 lanes finished: transpose to PSUM partition 0 (walrus
            # requires it), stage to SBUF, DMA-scatter to lane rows.
            # Bias is folded later via per-partition AP scalars.
            sl = slice(32 * s, 32 * (s + 1))
            ps_t = psum.tile([32, 128], F32, tag="pslab")
            nc.tensor.transpose(ps_t[:], y_all[:, sl], ident[:])
            stg = xpool.tile([32, 128], F32, tag="stg", bufs=2)
            nc.scalar.activation(stg[:], ps_t[:], Act.Identity, scale=1.0)
            nc.sync.dma_start(ys[sl, :], stg[:])

    # ---- d bits + mask precursors, one full-width pass after the stream ----
    nc.vector.tensor_scalar(dbits[:], ys[:], nbt[:], None, op0=Alu.is_gt)
    db = dbits[:].rearrange("p (b e) -> p b e", e=4)
    bq = [db[:, :, q:q + 1] for q in range(4)]
    nc.vector.tensor_tensor(c1[:], bq[0], bq[1], op=Alu.max)
    nc.vector.tensor_tensor(c2[:], c1[:], bq[2], op=Alu.max)
    nc.vector.tensor_tensor(c12[:], bq[1], bq[2], op=Alu.max)

    # ---- bootstrap 4-step maps: img[q] = first-one position >= q (else 0) ----
    # H tiles carry NBLK leading identity-map blocks so HS shifts slide into
    # identity instead of needing per-level copies of the unshifted prefix.
    HW_ = 2 * NBLK
    H0 = spool.tile([128, HW_ * PAD], F32)
    H1 = spool.tile([128, HW_ * PAD], F32)
    idmap = spool.tile([128, LANES_PER_SEQ * PAD], mybir.dt.int32)
    nc.gpsimd.iota(
        idmap[:].rearrange("p (b e) -> p b e", e=PAD)[:, :, 0:4],
        pattern=[[0, LANES_PER_SEQ], [1, 4]], channel_multiplier=0)
    for Ht in (H0, H1):
        nc.gpsimd.tensor_copy(
            _pv(Ht, NBLK),
            idmap[:].rearrange("p (b e) -> p b e", e=PAD)[:, :NBLK, 0:4])
    t1 = spool.tile([128, NBLK], F32)
    t2 = spool.tile([128, NBLK], F32)
    hv = H0[:].rearrange("p (b e) -> p b e", e=PAD)[:, NBLK:, :]
    i3, i2, i1, i0 = (hv[:, :, j:j + 1] for j in (3, 2, 1, 0))
    nc.vector.tensor_scalar(i3, bq[3], 3.0, None, op0=Alu.mult)
    nc.vector.scalar_tensor_tensor(out=t2[:], in0=i3, scalar=2.0, in1=bq[2],
                                   op0=Alu.subtract, op1=Alu.mult)
    nc.vector.tensor_tensor(i2, i3, t2[:], op=Alu.subtract)
    nc.vector.scalar_tensor_tensor(out=t2[:], in0=i2, scalar=1.0, in1=bq[1],
                                   op0=Alu.subtract, op1=Alu.mult)
    nc.vector.tensor_tensor(i1, i2, t2[:], op=Alu.subtract)
    nc.vector.tensor_tensor(t2[:], i1, bq[0], op=Alu.mult)
    nc.vector.tensor_tensor(i0, i1, t2[:], op=Alu.subtract)

    # ---- in-lane Hillis-Steele over 32 blocks (inclusive prefixes) ----
    cmp_s = spool.tile([128, NBLK * PAD], U8)
    cur, nxt = H0, H1
    for k in (1, 2, 4, 8, 16):
        cv = _pv(nxt, HW_)[:, NBLK:, :]
        av = _pv(cur, HW_)[:, NBLK:, :]
        bv = _pv(cur, HW_)[:, NBLK - k:HW_ - k, :]
        _combine(nc, cmp_s, cv, av, bv, 128, NBLK)
        cur, nxt = nxt, cur
    Hf = cur                                   # inclusive prefixes at [NBLK:]

    # ---- cross-lane HS over the 64 chunk summaries of each sequence ----
    M = LANES_PER_SEQ
    Sc = spool.tile([2, M * 4], F32)
    for s in range(SEQ_PER_CORE):
        nc.sync.dma_start(
            Sc[s:s + 1, :],
            Hf[64 * s:64 * (s + 1), (HW_ - 1) * PAD:(HW_ - 1) * PAD + 4])
    TW = 2 * M
    T0 = spool.tile([2, TW * PAD], F32)
    T1 = spool.tile([2, TW * PAD], F32)
    cmp2 = spool.tile([2, TW * PAD], U8)
    idm2 = idmap[:].rearrange("p (b e) -> p b e", e=PAD)[0:2, :M, 0:4]
    for Tt in (T0, T1):
        nc.gpsimd.tensor_copy(_pv(Tt, M)[:2], idm2)
    nc.vector.tensor_copy(_pv(T0, TW)[:2, M:, :],
                          Sc[:].rearrange("p (b e) -> p b e", e=4))
    cur, nxt = T0, T1
    for k in (1, 2, 4, 8, 16, 32):
        cv = _pv(nxt, TW)[:2, M:, :]
        av = _pv(cur, TW)[:2, M:, :]
        bv = _pv(cur, TW)[:2, M - k:TW - k, :]
        _combine(nc, cmp2, cv, av, bv, 2, M)
        cur, nxt = nxt, cur
    Sf = cur                                   # inclusive chunk prefixes

    # lane-entry STATES (entry 0 only): lane l gets inclusive[l-1][0]; lane 0 -> 0
    El0 = spool.tile([128, 1], F32)
    nc.gpsimd.memset(El0[:], 0.0)
    for s in range(SEQ_PER_CORE):
        nc.sync.dma_start(
            El0[64 * s + 1:64 * (s + 1), 0:1],
            _pv(Sf, TW)[s:s + 1, M:TW - 1, 0:1])

    # ---- apply: block-entry state per 4-block: sblk[b] = HfExcl[b][El0] ----
    sblk = spool.tile([128, NBLK], F32)
    nc.vector.tensor_copy(sblk[:, 0:1], El0[:])
    av = _pv(Hf, HW_)[:, NBLK:HW_ - 1, :]      # in-lane inclusive, blocks 0..30
    cm1 = spool.tile([128, 1], U8)
    nc.vector.tensor_copy(sblk[:, 1:NBLK], av[:, :, 0:1])   # El0==0 case
    for j in (1, 2, 3):
        nc.vector.tensor_scalar(cm1[:], El0[:], float(j), None, op0=Alu.is_equal)
        nc.vector.copy_predicated(sblk[:, 1:NBLK],
                                  cm1[:].broadcast_to([128, NBLK - 1]),
                                  av[:, :, j:j + 1])

    # ---- per-position masks from block-entry state s and bits ----
    mask = spool.tile([128, CHUNK], F32)
    mv = mask[:].rearrange("p (b e) -> p b e", e=4)
    mq = [mv[:, :, q:q + 1] for q in range(4)]
    e0 = spool.tile([128, NBLK], F32)
    e1 = spool.tile([128, NBLK], F32)
    e2 = spool.tile([128, NBLK], F32)
    u = spool.tile([128, NBLK], F32)
    u2 = spool.tile([128, NBLK], F32)
    nc.vector.tensor_scalar(e0[:], sblk[:], 0.0, None, op0=Alu.is_equal)
    nc.vector.tensor_scalar(e1[:], sblk[:], 1.0, None, op0=Alu.is_equal)
    nc.vector.tensor_scalar(e2[:], sblk[:], 2.0, None, op0=Alu.is_equal)
    nc.vector.tensor_scalar(mq[0], sblk[:], 0.0, None, op0=Alu.is_gt)
    # m1 = max(s>1, e0*b0)
    nc.vector.tensor_tensor(u[:], e0[:], bq[0], op=Alu.mult)
    nc.vector.scalar_tensor_tensor(out=mq[1], in0=sblk[:], scalar=1.0,
                                   in1=u[:], op0=Alu.is_gt, op1=Alu.max)
    # m2 = max(s>2, e0*c1, e1*b1)
    nc.vector.tensor_tensor(u[:], e0[:], c1[:], op=Alu.mult)
    nc.vector.scalar_tensor_tensor(out=mq[2], in0=sblk[:], scalar=2.0,
                                   in1=u[:], op0=Alu.is_gt, op1=Alu.max)
    nc.vector.tensor_tensor(u[:], e1[:], bq[1], op=Alu.mult)
    nc.vector.tensor_tensor(mq[2], mv[:, :, 2:3], u[:], op=Alu.max)
    # m3 = max(e0*c2, e1*c12, e2*b2)
    nc.vector.tensor_tensor(u[:], e0[:], c2[:], op=Alu.mult)
    nc.vector.tensor_tensor(u2[:], e1[:], c12[:], op=Alu.mult)
    nc.vector.tensor_tensor(mq[3], u[:], u2[:], op=Alu.max)
    nc.vector.tensor_tensor(u[:], e2[:], bq[2], op=Alu.mult)
    nc.vector.tensor_tensor(mq[3], mv[:, :, 3:4], u[:], op=Alu.max)

    # ---- epilogue ----
    dm = spool.tile([128, CHUNK], F32)
    nc.vector.scalar_tensor_tensor(
        out=dm[:], in0=mask[:], scalar=NEG, in1=ys[:],
        op0=Alu.mult, op1=Alu.add)
    bnd = spool.tile([128, CHUNK], F32)
    nc.vector.tensor_scalar(bnd[:], dm[:], nbt[:], None, op0=Alu.is_gt)
    ab = spool.tile([128, CHUNK], F32)
    nc.scalar.activation(ab[:], dm[:], Act.Abs, bias=bt[:])
    ex = spool.tile([128, CHUNK], F32)
    nc.scalar.activation(ex[:], ab[:], Act.Exp, scale=-1.0 / TAU)
    lg = spool.tile([128, CHUNK], F32)
    nc.scalar.activation(lg[:], ex[:], Act.Ln, bias=1.0)
    gth = spool.tile([128, CHUNK], F32)
    nc.scalar.activation(gth[:], lg[:], Act.Copy, scale=-1.0)

    bdst = bnd_out.rearrange("s (c w) -> (s c) w", w=CHUNK)
    gdst = gth_out.rearrange("s (c w) -> (s c) w", w=CHUNK)
    nc.sync.dma_start(bdst, bnd[:])
    nc.sync.dma_start(gdst, gth[:])


def build_program():
    nc = bacc.Bacc()
    x_in = nc.declare_dram_parameter("x", [ROWS, D], F32, isOutput=False)
    w_in = nc.declare_dram_parameter("w", [128, D], F32, isOutput=False)
    b_in = nc.declare_dram_parameter("bias", [128, 1], F32, isOutput=False)
    bnd_out = nc.declare_dram_parameter("bnd", [SEQ_PER_CORE, L], F32, isOutput=True)
    gth_out = nc.declare_dram_parameter("gth", [SEQ_PER_CORE, L], F32, isOutput=True)
    with tile.TileContext(nc) as tc:
        _program(tc, x_in[:], w_in[:], b_in[:], bnd_out[:], gth_out[:])
    nc.compile()
    return nc


_NC_CACHE = None


def kernel(x, label, W, b, _trace=False, _tmpdir=None):
    global _NC_CACHE
    x = np.ascontiguousarray(np.asarray(x, dtype=np.float32))
    W = np.asarray(W, dtype=np.float32)
    b = np.asarray(b, dtype=np.float32)
    wd = np.ascontiguousarray(np.repeat((W[:, 1] - W[:, 0])[None, :], 128, axis=0))
    bd = np.full((128, 1), np.float32(b[1] - b[0]), dtype=np.float32)

    if _NC_CACHE is None:
        _NC_CACHE = build_program()
    nc = _NC_CACHE

    in_maps = []
    for c in range(N_CORES):
        shard = x[c * SEQ_PER_CORE:(c + 1) * SEQ_PER_CORE].reshape(ROWS, D)
        in_maps.append({"x": np.ascontiguousarray(shard), "w": wd, "bias": bd})

    res = run_bass_kernel_spmd(nc, in_maps, list(range(N_CORES)),
                               trace=_trace, tmpdir=_tmpdir)
    boundaries = np.concatenate(
        [res.results[c]["bnd"] for c in range(N_CORES)], axis=0)
    gathered = np.concatenate(
        [res.results[c]["gth"] for c in range(N_CORES)], axis=0)[..., None]
    out = (boundaries.astype(np.float32), gathered.astype(np.float32))
    if _trace:
        return out, res
    return out
